# revision 9
# baseline (speedup 1.0000x reference)
"""Trainium2 Bass kernel for the mixture log-likelihood loss.

reference:
    log_otu = log(otu_dist + EPS)                       # (K=64, O=1024)
    lse[n,k] = counts[n] . log_otu[k] + log(comm+EPS)[k]
    out = sum_n logsumexp_k(lse[n, :])

Strategy (8 NeuronCores, data-parallel over N):
  * counts rows are small integers -> exact in fp8 e4m3. Cast on host,
    quartering HBM traffic (the kernel is memory-bound on counts). Falls
    back to an identically-structured bf16 module if the data ever stops
    being fp8-exact.
  * log_otu is split hi/lo into two fp8 (or bf16) tensors; the matmul
    streams each counts element once against a [hi | lo] 128-wide moving
    operand, accumulating both contributions in fp32 PSUM.
  * Two 128-particle blocks share one PSUM tile (128, 256) so the vector
    engine combines hi+lo+prior at (128, 2, 64) granularity, halving the
    per-op fixed cost. reduce_max(negate=True) feeds exp's per-partition
    bias; exp writes bf16 so the batched sum-exp reduce runs in 2x mode.
  * All Ln work is deferred to a single end-of-kernel activation over the
    (128, 98) gathered sums (avoids exp/ln ACT-table ping-pong).
  * Per-core partial sum is reduced over partitions with a tiny f32
    matmul against ones; the host adds the 8 scalars and analytically
    removes the zero-row padding contribution.
"""

import numpy as np
import ml_dtypes

N, K, O = 100000, 64, 1024
EPS = 1e-6
CORES = 8
NSHARD = N // CORES          # 12500
BLK = 128
NBLK = 98                    # ceil(12500 / 128)
NPAD = NBLK * BLK            # 12544
BPS = 14                     # blocks per superblock (even: pairs don't split)
SBS = NBLK // BPS            # 7 superblocks
PAD_ROWS = NPAD - NSHARD     # 44 zero rows per core

_cache = {}


def _build_module(use_fp8):
    import concourse.bacc as bacc
    import concourse.tile as tile
    from concourse import mybir

    f32 = mybir.dt.float32
    bf16 = mybir.dt.bfloat16
    cdt = mybir.dt.float8e4 if use_fp8 else bf16
    AX = mybir.AxisListType.X
    AF = mybir.ActivationFunctionType

    nc = bacc.Bacc("TRN2", target_bir_lowering=False, debug=False,
                   num_devices=CORES)
    cnts = nc.dram_tensor("cnts", [SBS, 128, BPS * 8 * BLK], cdt,
                          kind="ExternalInput").ap()
    hilo = nc.dram_tensor("hilo", [128, 8 * BLK], cdt,
                          kind="ExternalInput").ap()
    # prior duplicated for the 2-block batch: (128, [64 | 64])
    prior = nc.dram_tensor("prior", [128, 2, K], f32,
                           kind="ExternalInput").ap()
    ones = nc.dram_tensor("ones", [128, 1], f32, kind="ExternalInput").ap()
    out = nc.dram_tensor("out", [1, 1], f32, kind="ExternalOutput").ap()

    with tile.TileContext(nc, num_cores=CORES) as tc:
        with (
            tc.tile_pool(name="const", bufs=1) as const,
            tc.tile_pool(name="cnt", bufs=5) as cnt_pool,
            tc.tile_pool(name="work", bufs=6) as work,
            tc.tile_pool(name="psum", bufs=6, space="PSUM") as psum_pool,
            tc.tile_pool(name="fpsum", bufs=1, space="PSUM") as fpsum_pool,
        ):
            # constants ride the SWDGE queue so the big counts DMAs own HWDGE
            hilo_sb = const.tile([128, 8 * BLK], cdt)
            nc.gpsimd.dma_start(out=hilo_sb[:], in_=hilo)
            prior_sb = const.tile([128, 2, K], f32)
            nc.gpsimd.dma_start(out=prior_sb[:], in_=prior)
            ones_sb = const.tile([128, 1], f32)
            nc.gpsimd.dma_start(out=ones_sb[:], in_=ones)
            mg_all = const.tile([128, NBLK], f32)
            sg_all = const.tile([128, NBLK], f32)
            # touch Exp and Ln once (into a slice that is later fully
            # overwritten, so DCE keeps it) so both ACT table loads overlap
            # the DMA-bound head instead of landing in the kernel tail
            warm = const.tile([1, 1], f32)
            nc.vector.memset(warm[:], 1.0)
            nc.scalar.activation(sg_all[0:1, 0:1], warm[:], AF.Exp)
            nc.scalar.activation(sg_all[0:1, 0:1], warm[:], AF.Ln)

            PAIRW = 2 * 8 * BLK                     # cols per block pair
            for s in range(SBS):
                cnt = cnt_pool.tile([128, BPS * 8 * BLK], cdt)
                if s == 0:
                    # pair-granular loads: first matmul starts after 256KB
                    for p in range(BPS // 2):
                        nc.sync.dma_start(
                            out=cnt[:, p * PAIRW:(p + 1) * PAIRW],
                            in_=cnts[s, :, p * PAIRW:(p + 1) * PAIRW])
                else:
                    cut = 4 * PAIRW
                    nc.sync.dma_start(out=cnt[:, :cut], in_=cnts[s, :, :cut])
                    nc.sync.dma_start(out=cnt[:, cut:], in_=cnts[s, :, cut:])
                for p in range(BPS // 2):           # block pairs
                    gi = s * BPS + 2 * p            # global block index (even)
                    B2 = psum_pool.tile([128, 2, 2, K], mybir.dt.float32)
                    for h in range(2):              # block within pair
                        b = 2 * p + h
                        for c in range(8):
                            lo = (b * 8 + c) * BLK
                            nc.tensor.matmul(
                                B2[:, h, :, :],
                                lhsT=cnt[:, lo:lo + BLK],
                                rhs=hilo_sb[:, c * BLK:(c + 1) * BLK],
                                start=(h == 0 and c == 0),
                                stop=(h == 1 and c == 7),
                                skip_group_check=True,
                            )
                    # t2 = B2_hi + prior ; t2 += B2_lo    (128, (2,64))
                    t2 = work.tile([128, 2, K], mybir.dt.float32)
                    nc.vector.tensor_add(t2[:], B2[:, :, 0, :], prior_sb[:])
                    nc.vector.tensor_add(t2[:], t2[:], B2[:, :, 1, :])
                    nc.vector.reduce_max(mg_all[:, gi:gi + 2], t2[:],
                                         axis=AX, negate=True)
                    e2 = work.tile([128, 2, K], bf16)
                    for h in range(2):
                        nc.scalar.activation(e2[:, h, :], t2[:, h, :], AF.Exp,
                                             bias=mg_all[:, gi + h:gi + h + 1],
                                             scale=1.0)
                    nc.vector.reduce_sum(sg_all[:, gi:gi + 2], e2[:], axis=AX)

            ls = const.tile([128, NBLK], f32)
            nc.scalar.activation(ls[:], sg_all[:], AF.Ln)
            t3 = const.tile([128, NBLK], f32)
            nc.vector.tensor_sub(t3[:], ls[:], mg_all[:])
            accp = const.tile([128, 1], f32)
            nc.vector.reduce_sum(accp[:], t3[:], axis=AX)
            fin_ps = fpsum_pool.tile([1, 1], f32)
            nc.tensor.matmul(fin_ps[:], lhsT=accp[:], rhs=ones_sb[:],
                             start=True, stop=True)
            fin_sb = const.tile([1, 1], f32)
            nc.scalar.copy(fin_sb[:], fin_ps[:])
            nc.sync.dma_start(out=out, in_=fin_sb[:])

    nc.finalize()
    return nc


def _split2(x32, np_dt):
    hi = x32.astype(np_dt)
    lo = (x32 - hi.astype(np.float32)).astype(np_dt)
    return hi, lo


def _prep_inputs(counts, otu_dist, comm_dist, use_fp8):
    np_dt = ml_dtypes.float8_e4m3 if use_fp8 else ml_dtypes.bfloat16
    log_otu = np.log(otu_dist.astype(np.float32) + np.float32(EPS))
    hi, lo = _split2(log_otu, np_dt)
    # [p, c, k] = x[k, c*128 + p]
    hi_t = hi.reshape(K, 8, BLK).transpose(2, 1, 0)
    lo_t = lo.reshape(K, 8, BLK).transpose(2, 1, 0)
    hilo = np.ascontiguousarray(
        np.concatenate([hi_t, lo_t], axis=2)).reshape(128, 8 * BLK)

    prior_vec = np.log(comm_dist.astype(np.float32) + np.float32(EPS))
    prior = np.ascontiguousarray(
        np.broadcast_to(np.tile(prior_vec, 2)[None, :],
                        (128, 2 * K))).astype(np.float32).reshape(128, 2, K)
    ones = np.ones((128, 1), np.float32)

    counts_q = counts.astype(np_dt)
    shards = []
    for i in range(CORES):
        sh = counts_q[i * NSHARD:(i + 1) * NSHARD]
        shp = np.zeros((NPAD, O), np_dt)
        shp[:NSHARD] = sh
        # (s, b, j, c, p) -> (s, p, b, c, j)
        arr = shp.reshape(SBS, BPS, BLK, 8, BLK).transpose(0, 4, 1, 3, 2)
        shards.append(np.ascontiguousarray(arr).reshape(SBS, 128,
                                                        BPS * 8 * BLK))

    in_maps = [
        {"cnts": shards[i], "hilo": hilo, "prior": prior, "ones": ones}
        for i in range(CORES)
    ]
    # per-particle value contributed by each all-zero padding row
    pad_val = _np_logsumexp(prior_vec.astype(np.float64))
    return in_maps, pad_val


def _np_logsumexp(v):
    m = np.max(v)
    return m + np.log(np.sum(np.exp(v - m)))


def kernel(counts, otu_dist, comm_dist):
    from concourse.bass_utils import run_bass_kernel_spmd

    counts = np.asarray(counts)
    fp8 = ml_dtypes.float8_e4m3
    use_fp8 = bool(
        np.array_equal(counts.astype(fp8).astype(np.float32),
                       counts.astype(np.float32)))

    key = ("nc", use_fp8)
    if key not in _cache:
        _cache[key] = _build_module(use_fp8)
    nc = _cache[key]

    in_maps, pad_val = _prep_inputs(counts, np.asarray(otu_dist),
                                    np.asarray(comm_dist), use_fp8)
    res = run_bass_kernel_spmd(nc, in_maps, list(range(CORES)))
    total = sum(float(res.results[c]["out"][0, 0]) for c in range(CORES))
    total -= CORES * PAD_ROWS * pad_val
    return np.float32(total)


# revision 10
# speedup vs baseline: 1.0086x; 1.0086x over previous
"""Trainium2 Bass kernel for the mixture log-likelihood loss.

reference:
    log_otu = log(otu_dist + EPS)                       # (K=64, O=1024)
    lse[n,k] = counts[n] . log_otu[k] + log(comm+EPS)[k]
    out = sum_n logsumexp_k(lse[n, :])

Strategy (8 NeuronCores, data-parallel over N):
  * counts rows are small integers -> exact in fp8 e4m3. Cast on host,
    quartering HBM traffic (the kernel is memory-bound on counts). Falls
    back to an identically-structured bf16 module if the data ever stops
    being fp8-exact.
  * log_otu is split hi/lo into two fp8 (or bf16) tensors; the matmul
    streams each counts element once against a [hi | lo] 128-wide moving
    operand, accumulating both contributions in fp32 PSUM.
  * Two 128-particle blocks share one PSUM tile (128, 256) so the vector
    engine combines hi+lo+prior at (128, 2, 64) granularity, halving the
    per-op fixed cost. reduce_max(negate=True) feeds exp's per-partition
    bias; exp writes bf16 so the batched sum-exp reduce runs in 2x mode.
  * All Ln work is deferred to a single end-of-kernel activation over the
    (128, 98) gathered sums (avoids exp/ln ACT-table ping-pong).
  * Per-core partial sum is reduced over partitions with a tiny f32
    matmul against ones; the host adds the 8 scalars and analytically
    removes the zero-row padding contribution.
"""

import numpy as np
import ml_dtypes

N, K, O = 100000, 64, 1024
EPS = 1e-6
CORES = 8
NSHARD = N // CORES          # 12500
BLK = 128
NBLK = 98                    # ceil(12500 / 128)
NPAD = NBLK * BLK            # 12544
BPS = 14                     # blocks per superblock (even: pairs don't split)
SBS = NBLK // BPS            # 7 superblocks
PAD_ROWS = NPAD - NSHARD     # 44 zero rows per core

_cache = {}


def _build_module(use_fp8):
    import concourse.bacc as bacc
    import concourse.tile as tile
    from concourse import mybir

    # Force all activations (Exp/Ln/Copy) onto the one ACT table set that
    # contains them all — otherwise every Exp<->Ln switch pays a ~1.3us
    # ACT_TABLE_LOAD. Other sets are blanked (positions kept so the
    # act_func_set_id -> act_info.json index mapping stays valid).
    if not getattr(bacc, "_act_tables_patched", False):
        _orig_get = bacc.get_activation_tables

        def _only_ln_exp(arch):
            tabs = _orig_get(arch)
            return {
                name: (fns if name == "natural_log_exp_and_others" else set())
                for name, fns in tabs.items()
            }

        bacc.get_activation_tables = _only_ln_exp
        bacc._act_tables_patched = True

    f32 = mybir.dt.float32
    bf16 = mybir.dt.bfloat16
    cdt = mybir.dt.float8e4 if use_fp8 else bf16
    AX = mybir.AxisListType.X
    AF = mybir.ActivationFunctionType

    nc = bacc.Bacc("TRN2", target_bir_lowering=False, debug=False,
                   num_devices=CORES)
    cnts = nc.dram_tensor("cnts", [SBS, 128, BPS * 8 * BLK], cdt,
                          kind="ExternalInput").ap()
    hilo = nc.dram_tensor("hilo", [128, 8 * BLK], cdt,
                          kind="ExternalInput").ap()
    # prior duplicated for the 2-block batch: (128, [64 | 64])
    prior = nc.dram_tensor("prior", [128, 2, K], f32,
                           kind="ExternalInput").ap()
    ones = nc.dram_tensor("ones", [128, 1], f32, kind="ExternalInput").ap()
    out = nc.dram_tensor("out", [1, 1], f32, kind="ExternalOutput").ap()

    with tile.TileContext(nc, num_cores=CORES) as tc:
        with (
            tc.tile_pool(name="const", bufs=1) as const,
            tc.tile_pool(name="cnt", bufs=5) as cnt_pool,
            tc.tile_pool(name="work", bufs=6) as work,
            tc.tile_pool(name="psum", bufs=6, space="PSUM") as psum_pool,
            tc.tile_pool(name="fpsum", bufs=1, space="PSUM") as fpsum_pool,
        ):
            # constants ride the SWDGE queue so the big counts DMAs own HWDGE
            hilo_sb = const.tile([128, 8 * BLK], cdt)
            nc.gpsimd.dma_start(out=hilo_sb[:], in_=hilo)
            prior_sb = const.tile([128, 2, K], f32)
            nc.gpsimd.dma_start(out=prior_sb[:], in_=prior)
            ones_sb = const.tile([128, 1], f32)
            nc.gpsimd.dma_start(out=ones_sb[:], in_=ones)
            mg_all = const.tile([128, NBLK], f32)
            sg_all = const.tile([128, NBLK], f32)
            # touch Exp and Ln once (into a slice that is later fully
            # overwritten, so DCE keeps it) so both ACT table loads overlap
            # the DMA-bound head instead of landing in the kernel tail
            warm = const.tile([1, 1], f32)
            nc.vector.memset(warm[:], 1.0)
            nc.scalar.activation(sg_all[0:1, 0:1], warm[:], AF.Exp)
            nc.scalar.activation(sg_all[0:1, 0:1], warm[:], AF.Ln)

            PAIRW = 2 * 8 * BLK                     # cols per block pair
            for s in range(SBS):
                cnt = cnt_pool.tile([128, BPS * 8 * BLK], cdt)
                if s == 0:
                    # pair-granular loads: first matmul starts after 256KB
                    for p in range(BPS // 2):
                        nc.sync.dma_start(
                            out=cnt[:, p * PAIRW:(p + 1) * PAIRW],
                            in_=cnts[s, :, p * PAIRW:(p + 1) * PAIRW])
                else:
                    cut = 4 * PAIRW
                    nc.sync.dma_start(out=cnt[:, :cut], in_=cnts[s, :, :cut])
                    nc.sync.dma_start(out=cnt[:, cut:], in_=cnts[s, :, cut:])
                for p in range(BPS // 2):           # block pairs
                    gi = s * BPS + 2 * p            # global block index (even)
                    B2 = psum_pool.tile([128, 2, 2, K], mybir.dt.float32)
                    for h in range(2):              # block within pair
                        b = 2 * p + h
                        for c in range(8):
                            lo = (b * 8 + c) * BLK
                            nc.tensor.matmul(
                                B2[:, h, :, :],
                                lhsT=cnt[:, lo:lo + BLK],
                                rhs=hilo_sb[:, c * BLK:(c + 1) * BLK],
                                start=(h == 0 and c == 0),
                                stop=(h == 1 and c == 7),
                                skip_group_check=True,
                            )
                    # t2 = B2_hi + prior ; t2 += B2_lo    (128, (2,64))
                    t2 = work.tile([128, 2, K], mybir.dt.float32)
                    nc.vector.tensor_add(t2[:], B2[:, :, 0, :], prior_sb[:])
                    nc.vector.tensor_add(t2[:], t2[:], B2[:, :, 1, :])
                    nc.vector.reduce_max(mg_all[:, gi:gi + 2], t2[:],
                                         axis=AX, negate=True)
                    e2 = work.tile([128, 2, K], bf16)
                    for h in range(2):
                        nc.scalar.activation(e2[:, h, :], t2[:, h, :], AF.Exp,
                                             bias=mg_all[:, gi + h:gi + h + 1],
                                             scale=1.0)
                    nc.vector.reduce_sum(sg_all[:, gi:gi + 2], e2[:], axis=AX)

            ls = const.tile([128, NBLK], f32)
            nc.scalar.activation(ls[:], sg_all[:], AF.Ln)
            t3 = const.tile([128, NBLK], f32)
            nc.vector.tensor_sub(t3[:], ls[:], mg_all[:])
            accp = const.tile([128, 1], f32)
            nc.vector.reduce_sum(accp[:], t3[:], axis=AX)
            fin_ps = fpsum_pool.tile([1, 1], f32)
            nc.tensor.matmul(fin_ps[:], lhsT=accp[:], rhs=ones_sb[:],
                             start=True, stop=True)
            fin_sb = const.tile([1, 1], f32)
            nc.scalar.copy(fin_sb[:], fin_ps[:])
            nc.sync.dma_start(out=out, in_=fin_sb[:])

    nc.finalize()
    return nc


def _split2(x32, np_dt):
    hi = x32.astype(np_dt)
    lo = (x32 - hi.astype(np.float32)).astype(np_dt)
    return hi, lo


def _prep_inputs(counts, otu_dist, comm_dist, use_fp8):
    np_dt = ml_dtypes.float8_e4m3 if use_fp8 else ml_dtypes.bfloat16
    log_otu = np.log(otu_dist.astype(np.float32) + np.float32(EPS))
    hi, lo = _split2(log_otu, np_dt)
    # [p, c, k] = x[k, c*128 + p]
    hi_t = hi.reshape(K, 8, BLK).transpose(2, 1, 0)
    lo_t = lo.reshape(K, 8, BLK).transpose(2, 1, 0)
    hilo = np.ascontiguousarray(
        np.concatenate([hi_t, lo_t], axis=2)).reshape(128, 8 * BLK)

    prior_vec = np.log(comm_dist.astype(np.float32) + np.float32(EPS))
    prior = np.ascontiguousarray(
        np.broadcast_to(np.tile(prior_vec, 2)[None, :],
                        (128, 2 * K))).astype(np.float32).reshape(128, 2, K)
    ones = np.ones((128, 1), np.float32)

    counts_q = counts.astype(np_dt)
    shards = []
    for i in range(CORES):
        sh = counts_q[i * NSHARD:(i + 1) * NSHARD]
        shp = np.zeros((NPAD, O), np_dt)
        shp[:NSHARD] = sh
        # (s, b, j, c, p) -> (s, p, b, c, j)
        arr = shp.reshape(SBS, BPS, BLK, 8, BLK).transpose(0, 4, 1, 3, 2)
        shards.append(np.ascontiguousarray(arr).reshape(SBS, 128,
                                                        BPS * 8 * BLK))

    in_maps = [
        {"cnts": shards[i], "hilo": hilo, "prior": prior, "ones": ones}
        for i in range(CORES)
    ]
    # per-particle value contributed by each all-zero padding row
    pad_val = _np_logsumexp(prior_vec.astype(np.float64))
    return in_maps, pad_val


def _np_logsumexp(v):
    m = np.max(v)
    return m + np.log(np.sum(np.exp(v - m)))


def kernel(counts, otu_dist, comm_dist):
    from concourse.bass_utils import run_bass_kernel_spmd

    counts = np.asarray(counts)
    fp8 = ml_dtypes.float8_e4m3
    use_fp8 = bool(
        np.array_equal(counts.astype(fp8).astype(np.float32),
                       counts.astype(np.float32)))

    key = ("nc", use_fp8)
    if key not in _cache:
        _cache[key] = _build_module(use_fp8)
    nc = _cache[key]

    in_maps, pad_val = _prep_inputs(counts, np.asarray(otu_dist),
                                    np.asarray(comm_dist), use_fp8)
    res = run_bass_kernel_spmd(nc, in_maps, list(range(CORES)))
    total = sum(float(res.results[c]["out"][0, 0]) for c in range(CORES))
    total -= CORES * PAD_ROWS * pad_val
    return np.float32(total)
